# revision 1
# baseline (speedup 1.0000x reference)
"""NT-Xent / contrastive loss on 8 Trainium2 NeuronCores.

Reference computation (B=4096, D=512, temp=0.1):
    z   = l2norm(concat(proj_1, proj_2))          # [8192, 512]
    cos = (z @ z.T) / temp                        # [8192, 8192]
    pos[r]  = cos[r, (r + 4096) % 8192]
    lse[r]  = logsumexp(cos[r, :] with cos[r, r] masked out)
    loss    = mean(lse - pos)

Sharding: rows of the similarity matrix, 1024 per core.  Each core
receives the full stacked [8192, 512] input *rotated* by core*1024 rows,
which makes the program uniform across cores (SPMD): local rows 0..1023
are the core's rows, the self-diagonal sits at local column == row, and
the positive sits at local column == row + 4096.

Per core (measured ~238 us on TRN2, vs ~110 us bf16-GEMM roofline):
  1. Stream 64 row-tiles [128, 512] in; compute 1/||row|| with a fused
     square+row-sum (DVE scalar_tensor_tensor) and a fast-rsqrt +
     2 Newton steps (DVE integer magic, no ScalarE table traffic).
  2. Normalize + downcast each tile in one ScalarE Copy with a
     per-partition scale AP (rnorm), then transpose the bf16 tile with
     real PE matmuls against a constant bf16 identity; evacuate PSUM
     as bf16 into a [128, 4*8192] zT buffer (4 K-chunks of 128).
  3. GEMM: for each 128-row block m and 1024-col group J, accumulate
     8 bf16 matmuls (2 column chunks x 4 K-chunks) into a 2-bank PSUM
     tile, then a single ScalarE Exp(scale=1/temp) with accum_out
     produces the row-group sumexp.  The self/positive diagonals are
     pulled out of raw PSUM with a multiply-by-identity reduce before
     the in-place Exp (J==0 holds the self diagonal, J==4 the positive,
     both at column offset m*128, thanks to the input rotation).
  4. lse = ln(sumexp - exp(self*10)); partial = sum(lse - 10*pos) over
     the core's 1024 rows, reduced to a [1,1] scalar via a ones-matmul.
Host adds the 8 partials and divides by 8192.

Known HW quirks honored here: tensor_tensor_reduce does not execute on
this HW path (use scalar_tensor_tensor with accum_out instead);
scalar_tensor_tensor does not lower on GpSimd; fp32 matmuls double-pump
(HI/LO) so all GEMM operands are bf16; ScalarE Copy/Exp/Ln share one
activation table set (2 ACT_TABLE_LOADs total, no thrash).
"""

import sys

import numpy as np

if "/opt/trn_rl_repo" not in sys.path:
    sys.path.insert(0, "/opt/trn_rl_repo")

_B = 4096
_D = 512
_N2 = 2 * _B            # 8192 rows of the similarity matrix
_NCORES = 8
_RPC = _N2 // _NCORES   # 1024 rows per core
_INV_TEMP = 10.0

_NT = _N2 // 128        # 64 input row-tiles
_GRP = 8                # rsqrt batching: 8 tiles per group
_NM = _RPC // 128       # 8 output row blocks per core
_NJ = _N2 // 512        # 16 column chunks
_NK = _D // 128         # 4 contraction chunks

_MAGIC1 = 0x5F3759E0    # fast inverse sqrt magic + 1 (M - x == (M+1) + ~x)


def _emit(tc, projs, out_partial):
    import concourse.bass as bass  # noqa: F401
    from concourse import mybir

    nc = tc.nc
    f32 = mybir.dt.float32
    bf16 = mybir.dt.bfloat16
    i32 = mybir.dt.int32
    Alu = mybir.AluOpType
    Act = mybir.ActivationFunctionType

    from contextlib import ExitStack
    ctx = ExitStack()
    pool = ctx.enter_context(tc.tile_pool(name="work", bufs=1))
    pers = ctx.enter_context(tc.tile_pool(name="pers", bufs=1))
    pspool = ctx.enter_context(tc.tile_pool(name="psum", bufs=1, space="PSUM"))

    # ---- constants ----
    ones = pers.tile([128, 128], f32, tag="ones")
    nc.vector.memset(ones[:], 1.0)
    ident = pers.tile([128, 128], f32, tag="ident")
    nc.gpsimd.affine_select(ident[:], ones[:], pattern=[[1, 128]],
                            compare_op=Alu.is_equal, fill=0.0,
                            base=0, channel_multiplier=-1)
    identb = pers.tile([128, 128], bf16, tag="identb")
    nc.vector.tensor_copy(identb[:], ident[:])

    # ---- persistent buffers ----
    # zT, normalized, bf16: K-chunk k lives at columns [k*8192, (k+1)*8192).
    zt = pers.tile([128, _NK * _N2], bf16, tag="zt")
    zt3 = zt.rearrange("p (k c) -> p k c", k=_NK)
    sp_all = pers.tile([128, 2 * _NM], f32, tag="sp")    # self diag | pos diag
    rs_all = pers.tile([128, _NM], f32, tag="rs")        # row sumexp per block

    # ---- phase 1: load, norms, normalize (ScalarE) + transpose ----
    for g in range(_NT // _GRP):
        raws = []
        ss = pool.tile([128, _GRP], f32, tag="ss", bufs=2)
        for i in range(_GRP):
            t = g * _GRP + i
            raw = pool.tile([128, _D], f32, tag="raw", bufs=12,
                            name=f"raw{t}")
            nc.sync.dma_start(raw[:], projs[t * 128:(t + 1) * 128, :])
            raws.append(raw)
            sq = pool.tile([128, _D], bf16, tag="sq", bufs=2, name=f"sq{t}")
            nc.vector.scalar_tensor_tensor(
                out=sq[:], in0=raw[:], scalar=1.0, in1=raw[:],
                op0=Alu.mult, op1=Alu.mult, accum_out=ss[:, i:i + 1])

        # rnorm = 1/sqrt(max(ss, 1e-24)), fast-rsqrt + 2 Newton steps (DVE)
        ssc = pool.tile([128, _GRP], f32, tag="ssc", bufs=2, name=f"ssc{g}")
        nc.vector.tensor_scalar_max(ssc[:], ss[:], 1e-24)
        ti = pool.tile([128, _GRP], i32, tag="ti", bufs=2, name=f"ti{g}")
        nc.vector.tensor_scalar(
            out=ti[:], in0=ssc[:].bitcast(i32), scalar1=1, scalar2=-1,
            op0=Alu.logical_shift_right, op1=Alu.bitwise_xor)
        rn = pool.tile([128, _GRP], f32, tag="rn", bufs=2, name=f"rn{g}")
        nc.vector.tensor_scalar(
            out=rn[:].bitcast(i32), in0=ti[:], scalar1=_MAGIC1, scalar2=None,
            op0=Alu.add)
        nt = pool.tile([128, _GRP], f32, tag="nt", bufs=2, name=f"nt{g}")
        for _ in range(2):
            nc.vector.tensor_tensor(out=nt[:], in0=rn[:], in1=rn[:], op=Alu.mult)
            nc.vector.tensor_tensor(out=nt[:], in0=nt[:], in1=ssc[:], op=Alu.mult)
            nc.vector.tensor_scalar(out=nt[:], in0=nt[:], scalar1=-0.5,
                                    scalar2=1.5, op0=Alu.mult, op1=Alu.add)
            nc.vector.tensor_tensor(out=rn[:], in0=rn[:], in1=nt[:], op=Alu.mult)

        for i in range(_GRP):
            t = g * _GRP + i
            # normalize + bf16 downcast in one DVE op (per-partition scale);
            # keeps ScalarE free for the main-loop Exps so PSUM slots drain
            rawb = pool.tile([128, _D], bf16, tag="rawb", bufs=12,
                             name=f"rawb{t}")
            nc.vector.tensor_scalar_mul(rawb[:], raws[i][:], rn[:, i:i + 1])
            psT = pspool.tile([128, _D], f32, tag="psT", bufs=2,
                              name=f"psT{t}")
            for d in range(_NK):
                nc.tensor.matmul(psT[:, d * 128:(d + 1) * 128],
                                 rawb[:, d * 128:(d + 1) * 128],
                                 identb[:], start=True, stop=True)
            # one strided evacuation: [128, 4, 128] f32 -> bf16
            dst = zt3[:, :, t * 128:(t + 1) * 128]
            src = psT[:].rearrange("p (k c) -> p k c", k=_NK)
            nc.vector.tensor_copy(dst, src)

    # ---- phase 2: GEMM + exp + row sums (1024-wide exp groups) ----
    _NJG = _NJ // 2          # 8 groups of 2 512-chunks
    for m in range(_NM):
        se = pool.tile([128, _NJG], f32, tag="se", bufs=2, name=f"se{m}")
        off = m * 128
        for J in range(_NJG):
            ps = pspool.tile([128, 1024], f32, tag="ps", bufs=3,
                             name=f"ps{m}_{J}")
            for c in range(2):
                j = 2 * J + c
                for k in range(_NK):
                    nc.tensor.matmul(
                        ps[:, c * 512:(c + 1) * 512],
                        zt3[:, k, m * 128:(m + 1) * 128],
                        zt3[:, k, j * 512:(j + 1) * 512],
                        start=(k == 0), stop=(k == _NK - 1))
            if J == 0 or J == _NJG // 2:
                col = m if J == 0 else _NM + m
                junk = pool.tile([128, 128], f32, tag="junk", bufs=2,
                                 name=f"junk{m}_{J}")
                nc.vector.scalar_tensor_tensor(
                    out=junk[:], in0=ps[:, off:off + 128], scalar=1.0,
                    in1=ident[:], op0=Alu.mult, op1=Alu.mult,
                    accum_out=sp_all[:, col:col + 1])
            nc.scalar.activation(ps[:], ps[:], Act.Exp, bias=0.0,
                                 scale=_INV_TEMP, accum_out=se[:, J:J + 1])
        nc.vector.reduce_sum(out=rs_all[:, m:m + 1], in_=se[:],
                             axis=mybir.AxisListType.X)

    # ---- phase 3: lse, loss, partial sum ----
    sx = pool.tile([128, _NM], f32, tag="sx")
    nc.scalar.activation(sx[:], sp_all[:, 0:_NM], Act.Exp, bias=0.0,
                         scale=_INV_TEMP)
    nc.vector.tensor_sub(rs_all[:], rs_all[:], sx[:])
    lse = pool.tile([128, _NM], f32, tag="lse")
    nc.scalar.activation(lse[:], rs_all[:], Act.Ln, bias=0.0, scale=1.0)
    loss = pool.tile([128, _NM], f32, tag="loss")
    nc.vector.scalar_tensor_tensor(
        out=loss[:], in0=sp_all[:, _NM:2 * _NM], scalar=-_INV_TEMP,
        in1=lse[:], op0=Alu.mult, op1=Alu.add)
    lossv = pool.tile([128, 1], f32, tag="lossv")
    nc.vector.reduce_sum(out=lossv[:], in_=loss[:], axis=mybir.AxisListType.X)
    pf = pspool.tile([1, 1], f32, tag="psT", bufs=2)
    nc.tensor.matmul(pf[:], lossv[:], ones[:, 0:1], start=True, stop=True)
    res = pool.tile([1, 1], f32, tag="res")
    nc.vector.tensor_copy(res[:], pf[:])
    nc.sync.dma_start(out_partial[:, :], res[:])

    ctx.close()


def build():
    import concourse.tile as tile
    from concourse import bacc, mybir

    nc = bacc.Bacc("TRN2", target_bir_lowering=False, debug=False,
                   enable_asserts=True, num_devices=_NCORES)
    projs = nc.dram_tensor("projs", [_N2, _D], mybir.dt.float32,
                           kind="ExternalInput").ap()
    out_partial = nc.dram_tensor("partial", [1, 1], mybir.dt.float32,
                                 kind="ExternalOutput").ap()
    with tile.TileContext(nc) as tc:
        _emit(tc, projs, out_partial)
    nc.compile()
    return nc


_NC_CACHE = None


def _get_nc():
    global _NC_CACHE
    if _NC_CACHE is None:
        _NC_CACHE = build()
    return _NC_CACHE


def make_in_maps(proj_1, proj_2):
    z = np.concatenate([np.asarray(proj_1, dtype=np.float32),
                        np.asarray(proj_2, dtype=np.float32)], axis=0)
    return [{"projs": np.ascontiguousarray(np.roll(z, -_RPC * c, axis=0))}
            for c in range(_NCORES)]


def kernel(proj_1, proj_2):
    from concourse import bass_utils

    nc = _get_nc()
    in_maps = make_in_maps(proj_1, proj_2)
    r = bass_utils.run_bass_kernel_spmd(nc, in_maps,
                                        core_ids=list(range(_NCORES)))
    total = sum(float(res["partial"][0, 0]) for res in r.results)
    return np.float32(total / _N2)



# revision 3
# speedup vs baseline: 29.7638x; 29.7638x over previous
"""NT-Xent / contrastive loss on 8 Trainium2 NeuronCores (fp8 GEMM).

Reference computation (B=4096, D=512, temp=0.1):
    z   = l2norm(concat(proj_1, proj_2))          # [8192, 512]
    cos = (z @ z.T) / temp                        # [8192, 8192]
    pos[r]  = cos[r, (r + 4096) % 8192]
    lse[r]  = logsumexp(cos[r, :] with cos[r, r] masked out)
    loss    = mean(lse - pos)

Sharding: rows of the similarity matrix, 1024 per core.  Each core gets
the full stacked [8192, 512] input rotated by core*1024 rows (SPMD
uniform program; self-diagonal at local col == row, positive at local
col == row + 4096).

v2 design (vs the 238us bf16 baseline):
  - GEMM runs in fp8 e4m3 with MatmulPerfMode.DoubleRow (2 K-subtiles
    per instruction) = 2x bf16 PE throughput (~55us/core).
  - z is quantized as zq = 16 * l2norm(x) (fits e4m3 range/precision);
    psum = 256*cos, folded out via Exp scale 10/256.
  - Phase 1 per 128-row tile: dense f32 pair-loads, square+rowsum (DVE
    stt), fast-rsqrt (DVE int magic + 2 Newton), normalize+bf16
    downcast (DVE ts_mul by per-partition 16*rnorm), PE transpose
    against bf16 identity, PSUM->SBUF fp8 evacuation split between
    ScalarE (Copy) and DVE (tensor_copy) to balance engine load.
  - Phase 2 emitted interleaved with phase 1 per 8-tile group so PE/
    ScalarE/DVE/DMA all overlap: after group g's transposes, the GEMM
    for column group J=g runs for all 8 row blocks.
  - Row logsumexp: ScalarE Exp (in-place on PSUM, accum_out row sums)
    for 6 of 8 J-groups; the other 2 use a DVE Schraudolph exp
    (int32 bits = A*x + B, zero-mean-tuned constant) + reduce_sum to
    offload the exp-bound ScalarE.  Self/positive diagonals extracted
    from raw PSUM before the in-place Exp; self term subtracted with
    the identical ScalarE Exp pathway so it cancels exactly.
"""

import sys

import numpy as np

if "/opt/trn_rl_repo" not in sys.path:
    sys.path.insert(0, "/opt/trn_rl_repo")

_B = 4096
_D = 512
_N2 = 2 * _B            # 8192 rows of the similarity matrix
_NCORES = 8
_RPC = _N2 // _NCORES   # 1024 rows per core
_INV_TEMP = 10.0
_FSC = 16.0             # fp8 scale: zq = 16*z, psum = 256*cos
_ESC = _INV_TEMP / (_FSC * _FSC)   # Exp scale on psum

_NT = _N2 // 128        # 64 input row-tiles
_GRP = 8                # tiles per pipeline group
_NG = _NT // _GRP       # 8 groups
_NM = _RPC // 128       # 8 output row blocks per core
_NK = _D // 128         # 4 contraction chunks (2 DoubleRow pairs)

_MAGIC1 = 0x5F3759E0    # fast inverse sqrt magic + 1 (M - x == (M+1) + ~x)

# Schraudolph exp on psum values: float32 bits = A*x + B (zero-mean C)
_SCH_C = 0.05640058203329989
_SCH_A = float((2.0 ** 23) / np.log(2.0) * _ESC)
_SCH_B = float(127.0 * 2 ** 23 - _SCH_C * 2 ** 23)
_SCH_GROUPS = (5, 6)    # J-groups handled by DVE Schraudolph (even m only)


def _emit(tc, projs, out_partial):
    import concourse.bass as bass  # noqa: F401
    from concourse import mybir

    nc = tc.nc
    f32 = mybir.dt.float32
    bf16 = mybir.dt.bfloat16
    fp8 = mybir.dt.float8e4
    i32 = mybir.dt.int32
    Alu = mybir.AluOpType
    Act = mybir.ActivationFunctionType
    DR = mybir.MatmulPerfMode.DoubleRow

    from contextlib import ExitStack
    ctx = ExitStack()
    pool = ctx.enter_context(tc.tile_pool(name="work", bufs=1))
    pers = ctx.enter_context(tc.tile_pool(name="pers", bufs=1))
    pspool = ctx.enter_context(tc.tile_pool(name="psum", bufs=1, space="PSUM"))

    # ---- constants ----
    ones = pers.tile([128, 128], f32, tag="ones")
    nc.vector.memset(ones[:], 1.0)
    ident = pers.tile([128, 128], f32, tag="ident")
    nc.gpsimd.affine_select(ident[:], ones[:], pattern=[[1, 128]],
                            compare_op=Alu.is_equal, fill=0.0,
                            base=0, channel_multiplier=-1)
    identb = pers.tile([128, 128], bf16, tag="identb")
    nc.vector.tensor_copy(identb[:], ident[:])

    # ---- persistent buffers ----
    # zt fp8: K-chunk k at columns [k*8192, (k+1)*8192); [p, k, c] layout
    zt = pers.tile([128, _NK * _N2], fp8, tag="zt")
    zt3 = zt.rearrange("p (k c) -> p k c", k=_NK)
    sp_all = pers.tile([128, 2 * _NM], f32, tag="sp")    # self diag | pos diag
    se_all = pers.tile([128, _NM * _NG], f32, tag="se")  # per (m, J) row sums

    pv = projs.rearrange("(u two p) d -> u p two d", two=2, p=128)

    def phase1(g):
        raw2s = []
        ss = pool.tile([128, _GRP], f32, tag="ss", bufs=2, name=f"ss{g}")
        for u2 in range(_GRP // 2):
            u = g * (_GRP // 2) + u2
            raw2 = pool.tile([128, 2, _D], f32, tag="raw", bufs=12,
                             name=f"raw{u}")
            nc.sync.dma_start(raw2[:], pv[u])
            raw2s.append(raw2)
            for h in range(2):
                i = 2 * u2 + h
                t = g * _GRP + i
                sq = pool.tile([128, _D], bf16, tag="sq", bufs=2,
                               name=f"sq{t}")
                nc.vector.scalar_tensor_tensor(
                    out=sq[:], in0=raw2[:, h], scalar=1.0, in1=raw2[:, h],
                    op0=Alu.mult, op1=Alu.mult, accum_out=ss[:, i:i + 1])

        # rn16 = 16/sqrt(max(ss,1e-24)): fast-rsqrt + 2 Newton steps (DVE)
        ssc = pool.tile([128, _GRP], f32, tag="ssc", bufs=2, name=f"ssc{g}")
        nc.vector.tensor_scalar_max(ssc[:], ss[:], 1e-24)
        ti = pool.tile([128, _GRP], i32, tag="ti", bufs=2, name=f"ti{g}")
        nc.vector.tensor_scalar(
            out=ti[:], in0=ssc[:].bitcast(i32), scalar1=1, scalar2=-1,
            op0=Alu.logical_shift_right, op1=Alu.bitwise_xor)
        rn = pool.tile([128, _GRP], f32, tag="rn", bufs=2, name=f"rn{g}")
        nc.vector.tensor_scalar(
            out=rn[:].bitcast(i32), in0=ti[:], scalar1=_MAGIC1, scalar2=None,
            op0=Alu.add)
        nt = pool.tile([128, _GRP], f32, tag="nt", bufs=2, name=f"nt{g}")
        for _ in range(2):
            nc.vector.tensor_tensor(out=nt[:], in0=rn[:], in1=rn[:],
                                    op=Alu.mult)
            nc.vector.tensor_tensor(out=nt[:], in0=nt[:], in1=ssc[:],
                                    op=Alu.mult)
            nc.vector.tensor_scalar(out=nt[:], in0=nt[:], scalar1=-0.5,
                                    scalar2=1.5, op0=Alu.mult, op1=Alu.add)
            nc.vector.tensor_tensor(out=rn[:], in0=rn[:], in1=nt[:],
                                    op=Alu.mult)
        rn16 = pool.tile([128, _GRP], f32, tag="rn16", bufs=2,
                         name=f"rn16{g}")
        nc.vector.tensor_scalar(out=rn16[:], in0=rn[:], scalar1=_FSC,
                                scalar2=None, op0=Alu.mult)

        for i in range(_GRP):
            t = g * _GRP + i
            # normalize + bf16 downcast (per-partition scale)
            rawb = pool.tile([128, _D], bf16, tag="rawb", bufs=4,
                             name=f"rawb{t}")
            nc.vector.tensor_scalar_mul(rawb[:], raw2s[i // 2][:, i % 2],
                                        rn16[:, i:i + 1])
            psT = pspool.tile([128, _D], f32, tag="psT", bufs=2,
                              name=f"psT{t}")
            for d in range(_NK):
                nc.tensor.matmul(psT[:, d * 128:(d + 1) * 128],
                                 rawb[:, d * 128:(d + 1) * 128],
                                 identb[:], start=True, stop=True)
            # evacuate PSUM f32 -> zt fp8 (strided [p,k,c] dst); split
            # between ScalarE and DVE to balance load
            dst = zt3[:, :, t * 128:(t + 1) * 128]
            srcT = psT[:].rearrange("p (k c) -> p k c", k=_NK)
            if t % 3 == 0:
                nc.vector.tensor_copy(dst, srcT)
            else:
                nc.scalar.activation(dst, srcT, Act.Copy, bias=0.0, scale=1.0)

    def gemm_expcol(m, g):
        # GEMM row block m x column group J==g (1024 cols), then exp+rowsum
        ps = pspool.tile([128, 1024], f32, tag="ps", bufs=3,
                         name=f"ps{m}_{g}")
        for c in range(2):
            j = 2 * g + c
            for k2 in range(2):
                nc.tensor.matmul(
                    ps[:, c * 512:(c + 1) * 512],
                    zt3[:, 2 * k2:2 * k2 + 2, m * 128:(m + 1) * 128],
                    zt3[:, 2 * k2:2 * k2 + 2, j * 512:(j + 1) * 512],
                    start=(k2 == 0), stop=(k2 == 1), perf_mode=DR)
        off = m * 128
        if g == 0 or g == _NG // 2:
            col = m if g == 0 else _NM + m
            junk = pool.tile([128, 128], f32, tag="junk", bufs=2,
                             name=f"junk{m}_{g}")
            nc.vector.scalar_tensor_tensor(
                out=junk[:], in0=ps[:, off:off + 128], scalar=1.0,
                in1=ident[:], op0=Alu.mult, op1=Alu.mult,
                accum_out=sp_all[:, col:col + 1])
        secol = se_all[:, m * _NG + g:m * _NG + g + 1]
        if g in _SCH_GROUPS and m % 2 == 0:
            tmp = pool.tile([128, 1024], i32, tag="tmp", bufs=2,
                            name=f"tmp{m}_{g}")
            nc.vector.tensor_scalar(
                out=tmp[:], in0=ps[:], scalar1=_SCH_A, scalar2=_SCH_B,
                op0=Alu.mult, op1=Alu.add)
            nc.vector.reduce_sum(out=secol, in_=tmp[:].bitcast(f32),
                                 axis=mybir.AxisListType.X)
        else:
            nc.scalar.activation(ps[:], ps[:], Act.Exp, bias=0.0,
                                 scale=_ESC, accum_out=secol)

    # ---- pipelined emission: phase1(g) then GEMM column group g ----
    for g in range(_NG):
        phase1(g)
        for m in range(_NM):
            gemm_expcol(m, g)

    # ---- phase 3: lse, loss, partial sum ----
    rs = pool.tile([128, _NM], f32, tag="rs")
    se4 = se_all.rearrange("p (m j) -> p m j", m=_NM)
    nc.vector.tensor_reduce(out=rs[:], in_=se4, axis=mybir.AxisListType.X,
                            op=Alu.add)
    sx = pool.tile([128, _NM], f32, tag="sx")
    nc.scalar.activation(sx[:], sp_all[:, 0:_NM], Act.Exp, bias=0.0,
                         scale=_ESC)
    nc.vector.tensor_sub(rs[:], rs[:], sx[:])
    lse = pool.tile([128, _NM], f32, tag="lse")
    nc.scalar.activation(lse[:], rs[:], Act.Ln, bias=0.0, scale=1.0)
    loss = pool.tile([128, _NM], f32, tag="loss")
    nc.vector.scalar_tensor_tensor(
        out=loss[:], in0=sp_all[:, _NM:2 * _NM], scalar=-_ESC,
        in1=lse[:], op0=Alu.mult, op1=Alu.add)
    lossv = pool.tile([128, 1], f32, tag="lossv")
    nc.vector.reduce_sum(out=lossv[:], in_=loss[:], axis=mybir.AxisListType.X)
    pf = pspool.tile([1, 1], f32, tag="psT", bufs=2)
    nc.tensor.matmul(pf[:], lossv[:], ones[:, 0:1], start=True, stop=True)
    res = pool.tile([1, 1], f32, tag="res")
    nc.vector.tensor_copy(res[:], pf[:])
    nc.sync.dma_start(out_partial[:, :], res[:])

    ctx.close()


def build():
    import concourse.tile as tile
    from concourse import bacc, mybir

    nc = bacc.Bacc("TRN2", target_bir_lowering=False, debug=False,
                   enable_asserts=True, num_devices=_NCORES)
    projs = nc.dram_tensor("projs", [_N2, _D], mybir.dt.float32,
                           kind="ExternalInput").ap()
    out_partial = nc.dram_tensor("partial", [1, 1], mybir.dt.float32,
                                 kind="ExternalOutput").ap()
    with tile.TileContext(nc) as tc:
        _emit(tc, projs, out_partial)
    nc.compile()
    return nc


_NC_CACHE = None


def _get_nc():
    global _NC_CACHE
    if _NC_CACHE is None:
        _NC_CACHE = build()
    return _NC_CACHE


def make_in_maps(proj_1, proj_2):
    z = np.concatenate([np.asarray(proj_1, dtype=np.float32),
                        np.asarray(proj_2, dtype=np.float32)], axis=0)
    return [{"projs": np.ascontiguousarray(np.roll(z, -_RPC * c, axis=0))}
            for c in range(_NCORES)]


def kernel(proj_1, proj_2):
    from concourse import bass_utils

    nc = _get_nc()
    in_maps = make_in_maps(proj_1, proj_2)
    r = bass_utils.run_bass_kernel_spmd(nc, in_maps,
                                        core_ids=list(range(_NCORES)))
    total = sum(float(res["partial"][0, 0]) for res in r.results)
    return np.float32(total / _N2)
